# revision 1
# baseline (speedup 1.0000x reference)
"""Trainium2 Bass kernel for nn_Mhsa_47802986004933.

Model (per batch b of 2):
  BN(train-stats)+ReLU -> 1x1 conv qkv (raw .view reinterpret) ->
  4-head attention on heads 0-3  +  conv-mixing (3x1 / 1x3) on heads 4-7 ->
  concat -> kernel-2 avg pool.

Sharding: 8 cores = (batch b in {0,1}) x (h in {0..3}).
  Core c = 4b + h:
    - full 4096x4096 attention for head h of batch b  -> out[b, :, 32h:32h+32]
    - conv y-quarter [16h, 16h+16)                    -> out[b, n%16 in [4h,4h+4), 128:256]
  Communication-free SPMD: BN stats recomputed on every core from the full x.

Key structural identity: with O = W @ xn [1536, 4096] per batch and
U = O.reshape(12288, 512) (u = 8o+g), token n has q = U[3n], k = U[3n+1],
v = U[3n+2].  Attention head h uses U columns [64h, 64h+64); the conv branch
uses columns [256, 512) with image layout q2[i, y, x] =
U[3*(64*(i%64)+y), 256 + 64*(i//64) + x].

All matmuls run as float32r (f32 storage, 1 cycle/row on the PE at N>=256).
"""
import os
import sys
import numpy as np
import ml_dtypes

sys.path.insert(0, "/opt/trn_rl_repo")

import concourse.bass as bass
import concourse.bacc as bacc
import concourse.mybir as mybir
import concourse.tile as tile
from concourse import bass_utils

B, N, DIM, S = 2, 4096, 256, 64
H, DH, INNER = 8, 64, 512
EPS = 1e-5
FP = mybir.dt.float32
FR = mybir.dt.float32r
BF = mybir.dt.bfloat16
AF = mybir.ActivationFunctionType
OP = mybir.AluOpType

# scores exp groups per 512-query chunk: 32 key-blocks split (3,3,...,3,2)
GROUPS = [3, 2] * 6 + [2]


def _r(ap):
    return ap.bitcast(FR)


def build_device_program():
    nc = bacc.Bacc(
        "TRN2", target_bir_lowering=False, debug=False, enable_asserts=True,
        num_devices=8,
    )
    xts = nc.dram_tensor("xts", [256, 8192], BF, kind="ExternalInput").ap()
    xc_d = nc.dram_tensor("xc", [256, 2560], FP, kind="ExternalInput").ap()
    wq_d = nc.dram_tensor("wq", [256, 1536], FP, kind="ExternalInput").ap()
    wcg_d = nc.dram_tensor("wcg", [256, 3072], FP, kind="ExternalInput").ap()
    wch_d = nc.dram_tensor("wch", [256, 128], FP, kind="ExternalInput").ap()
    w1s_d = nc.dram_tensor("w1s", [256, 768], FP, kind="ExternalInput").ap()
    w2s_d = nc.dram_tensor("w2s", [256, 768], FP, kind="ExternalInput").ap()
    gb_d = nc.dram_tensor("gb", [256, 2], FP, kind="ExternalInput").ap()
    idn_d = nc.dram_tensor("idn", [128, 128], FP, kind="ExternalInput").ap()
    out_a = nc.dram_tensor("out_a", [4096, 32], FP, kind="ExternalOutput").ap()
    out_c = nc.dram_tensor("out_c", [1024, 128], FP, kind="ExternalOutput").ap()

    with tile.TileContext(nc) as tc:
        _emit(tc, nc, xts, xc_d, wq_d, wcg_d, wch_d, w1s_d, w2s_d, gb_d, idn_d,
              out_a, out_c)
    nc.compile()
    return nc


def _emit(tc, nc, xts, xc_d, wq_d, wcg_d, wch_d, w1s_d, w2s_d, gb_d, idn_d,
          out_a, out_c):
    from contextlib import ExitStack
    ctx = ExitStack()
    with ctx:
        cp = ctx.enter_context(tc.tile_pool(name="const", bufs=1))
        sctx = ExitStack()
        sp = sctx.enter_context(tc.tile_pool(name="scratch", bufs=1))
        xp = sctx.enter_context(tc.tile_pool(name="xload", bufs=2))
        pm = ctx.enter_context(tc.tile_pool(name="ps_m", bufs=2, space="PSUM"))
        pa_ = ctx.enter_context(tc.tile_pool(name="ps_a", bufs=1, space="PSUM"))
        pb_ = ctx.enter_context(tc.tile_pool(name="ps_b", bufs=1, space="PSUM"))
        po = ctx.enter_context(tc.tile_pool(name="ps_o", bufs=1, space="PSUM"))

        # ---------------- persistent SBUF ----------------
        wq = [cp.tile([128, 1536], FR, tag="wq0", name="wq0"),
              cp.tile([128, 1536], FR, tag="wq1", name="wq1")]
        w1s = [cp.tile([128, 768], FR, tag="w1s0", name="w1s0"),
               cp.tile([128, 768], FR, tag="w1s1", name="w1s1")]
        w2s = [cp.tile([128, 768], FR, tag="w2s0", name="w2s0"),
               cp.tile([128, 768], FR, tag="w2s1", name="w2s1")]
        gb = [cp.tile([128, 2], FP, tag="gb0", name="gb0"),
              cp.tile([128, 2], FP, tag="gb1", name="gb1")]
        idn = cp.tile([128, 128], FR, tag="idn", name="idn")
        idnf = cp.tile([128, 128], FP, tag="idnf", name="idnf")
        ut = cp.tile([64, 12288], FR, tag="ut", name="ut")
        vsb = cp.tile([128, 2080], FR, tag="vsb", name="vsb")
        sq2 = sp.tile([128, 2048], FR, tag="sq2", name="sq2")  # slim q (j=0)
        sk2 = sp.tile([128, 2048], FR, tag="sk2", name="sk2")
        sv2 = sp.tile([128, 2048], FR, tag="sv2", name="sv2")
        qhalo = [sp.tile([64, 256], FR, tag="qhalo0", name="qhalo0"),
                 sp.tile([64, 256], FR, tag="qhalo1", name="qhalo1")]
        q2q = [cp.tile([128, 1152], FR, tag="q2q0", name="q2q0"),
               cp.tile([128, 1152], FR, tag="q2q1", name="q2q1")]
        k2q = [cp.tile([128, 1024], FR, tag="k2q0", name="k2q0"),
               cp.tile([128, 1024], FR, tag="k2q1", name="k2q1")]
        v2q = [cp.tile([128, 1024], FR, tag="v2q0", name="v2q0"),
               cp.tile([128, 1024], FR, tag="v2q1", name="v2q1")]

        # transient loads (scratch pool, released before k2l/k2r + work pools)
        wch = [sp.tile([128, 128], FR, tag="wch0", name="wch0"),
               sp.tile([128, 128], FR, tag="wch1", name="wch1")]
        xcld = [cp.tile([128, 2560], FR, tag="xc0", name="xc0"),
                cp.tile([128, 2560], FR, tag="xc1", name="xc1")]
        bnst = [sp.tile([128, 96], FP, tag="bnst0", name="bnst0"),
                sp.tile([128, 96], FP, tag="bnst1", name="bnst1")]

        dma = nc.sync.dma_start
        vec = nc.vector
        act = nc.scalar

        zconst = cp.tile([128, 1], FP, tag="zconst", name="zconst")
        vec.memset(zconst, 0.0)
        nc.const_aps.aps[(FP, 0.0)] = zconst
        epst = cp.tile([128, 1], FP, tag="epst", name="epst")
        vec.memset(epst, EPS)
        onec = cp.tile([128, 1], FP, tag="onec", name="onec")
        vec.memset(onec, 1.0)

        # attention work tiles (declared before scratch release; manual 2x alt)
        psbs = [cp.tile([128, 1536], FR, tag="psbA", name="psbA"),
                cp.tile([128, 1536], FR, tag="psbB", name="psbB")]
        osbs = [cp.tile([65, 512], FP, tag="osbA", name="osbA"),
                cp.tile([65, 512], FP, tag="osbB", name="osbB")]
        resbs = [cp.tile([128, 128], FP, tag="resbA", name="resbA"),
                 cp.tile([128, 128], FP, tag="resbB", name="resbB")]
        trss = [cp.tile([128, 65], FP, tag="trsA", name="trsA"),
                cp.tile([128, 65], FP, tag="trsB", name="trsB")]
        recs = [cp.tile([128, 1], FP, tag="recA", name="recA"),
                cp.tile([128, 1], FP, tag="recB", name="recB")]
        pav2s = [cp.tile([128, 32], FP, tag="pav2A", name="pav2A"),
                 cp.tile([128, 32], FP, tag="pav2B", name="pav2B")]

        # ---------------- BN stats over full x (critical path: DMA first) ----
        mvs = []
        for hf in range(2):
            for ck in range(2):
                xk = xp.tile([128, 4096], BF, tag="xk", name=f"xk{hf}{ck}")
                dma(out=xk, in_=xts[128 * hf:128 * hf + 128,
                                    4096 * ck:4096 * ck + 4096])
                for sub in range(8):
                    kk = 8 * ck + sub
                    vec.bn_stats(out=bnst[hf][:, 6 * kk:6 * kk + 6],
                                 in_=xk[:, 512 * sub:512 * sub + 512])
            mv = sp.tile([128, 2], FP, tag="mv", bufs=2, name=f"mv{hf}")
            vec.bn_aggr(out=mv, in_=bnst[hf].rearrange("p (k s) -> p k s", s=6))
            mvs.append(mv)

        # ---------------- loads (xc/wq next on the critical path) ----------
        for hf in range(2):
            dma(out=xcld[hf], in_=xc_d.bitcast(FR)[128 * hf:128 * hf + 128, :])
            dma(out=gb[hf], in_=gb_d[128 * hf:128 * hf + 128, :])
        for hf in range(2):
            dma(out=wq[hf], in_=wq_d.bitcast(FR)[128 * hf:128 * hf + 128, :])
        for hf in range(2):
            dma(out=w1s[hf], in_=w1s_d.bitcast(FR)[128 * hf:128 * hf + 128, :])
            dma(out=w2s[hf], in_=w2s_d.bitcast(FR)[128 * hf:128 * hf + 128, :])
            dma(out=wch[hf], in_=wch_d.bitcast(FR)[128 * hf:128 * hf + 128, :])
        dma(out=idn, in_=idn_d.bitcast(FR))
        dma(out=idnf, in_=idn_d)

        # affine: a = gamma*rsqrt(var+eps); bb = beta - mean*a
        aff = []
        for hf in range(2):
            sqv = sp.tile([128, 1], FP, tag="sqv", bufs=2, name=f"sqv{hf}")
            act.activation(sqv, mvs[hf][:, 1:2], AF.Sqrt, bias=epst)
            rsv = sp.tile([128, 1], FP, tag="rsv", bufs=2, name=f"rsv{hf}")
            vec.reciprocal(rsv, sqv)
            a_ = sp.tile([128, 1], FP, tag="a_", bufs=2, name=f"a{hf}")
            vec.tensor_tensor(a_, rsv, gb[hf][:, 0:1], OP.mult)
            tmp = sp.tile([128, 1], FP, tag="tmp", bufs=2, name=f"tmp{hf}")
            vec.tensor_tensor(tmp, mvs[hf][:, 0:1], a_, OP.mult)
            bb = sp.tile([128, 1], FP, tag="bb", bufs=2, name=f"bb{hf}")
            vec.tensor_tensor(bb, gb[hf][:, 1:2], tmp, OP.subtract)
            aff.append((a_, bb))

        # xn = relu(a*x + b)   (ACT is idle this early; in-place on the load)
        xn = xcld
        for hf in range(2):
            a_, bb = aff[hf]
            act.activation(xn[hf], xn[hf], AF.Relu, bias=bb, scale=a_)

        # ---------------- head projection -> ut ----------------
        # ut[:, 8o+g] = OUT_g[:, o];  OUT_g = xn_head_g^T @ WT
        for g in range(8):
            for oc in range(3):
                ps = pm.tile([64, 512], FP, tag="m", name=f"pr{g}_{oc}")
                nc.tensor.matmul(ps, (xn[0][:, 64 * g:64 * g + 64]),
                                 (wq[0][:, 512 * oc:512 * oc + 512]),
                                 start=True, stop=False)
                nc.tensor.matmul(ps, (xn[1][:, 64 * g:64 * g + 64]),
                                 (wq[1][:, 512 * oc:512 * oc + 512]),
                                 start=False, stop=True)
                vec.tensor_copy(
                    ut[:, 4096 * oc + g: 4096 * oc + g + 4089: 8], ps)

        # ---------------- V tiles (token-major) + ones column ----------------
        vec.tensor_copy(vsb[:, 64::65], onec.to_broadcast((128, 32)))
        for t in range(32):
            pv = pm.tile([128, 64], FR, tag="m", name=f"vt{t}")
            nc.tensor.transpose(pv, ut[:, 3 * 128 * t + 2: 3 * 128 * t + 384: 3],
                                idn[0:64, 0:64])
            vec.tensor_copy(vsb[:, 65 * t:65 * t + 64], pv)

        ABL = ""  # ablation switch used only during development
        # ---------------- conv-input slim projection ----------------
        # sX[64*mr+ilo, 256*rho+f] = U[3*(8*(8*ilo+2h+mr)+rho)+j, 256+f]
        for j, dst in (() if ABL == "noconv" else ((0, sq2), (1, sk2), (2, sv2))):
            wcgj = [sp.tile([128, 1024], FR, tag="wcgj0", name=f"wcgj0_{j}"),
                    sp.tile([128, 1024], FR, tag="wcgj1", name=f"wcgj1_{j}")]
            for hf in range(2):
                dma(out=wcgj[hf], in_=wcg_d.bitcast(FR)[
                    128 * hf:128 * hf + 128, 1024 * j:1024 * j + 1024])
            for rho in range(8):
                g = (3 * rho + j) % 8
                ps = pm.tile([128, 256], FP, tag="m", name=f"pc{j}_{rho}")
                nc.tensor.matmul(ps, (wcgj[0][:, 128 * rho:128 * rho + 128]),
                                 (xn[0][:, 512 + 256 * g:512 + 256 * g + 256]),
                                 start=True, stop=False)
                nc.tensor.matmul(ps, (wcgj[1][:, 128 * rho:128 * rho + 128]),
                                 (xn[1][:, 512 + 256 * g:512 + 256 * g + 256]),
                                 start=False, stop=True)
                vec.tensor_copy(dst[:, 256 * rho:256 * rho + 256], ps)

        # halo rows (j=0): lo rho=7 g=5 ; hi rho=0 g=0  (separate 64-part tiles)
        for e, wcol, gg in (() if ABL == "noconv" else ((0, 0, 5), (1, 64, 0))):
            ph = pm.tile([64, 256], FP, tag="m", name=f"phalo{e}")
            nc.tensor.matmul(ph, (wch[0][:, wcol:wcol + 64]),
                             (xn[0][:, 512 + 256 * gg:512 + 256 * gg + 256]),
                             start=True, stop=False)
            nc.tensor.matmul(ph, (wch[1][:, wcol:wcol + 64]),
                             (xn[1][:, 512 + 256 * gg:512 + 256 * gg + 256]),
                             start=False, stop=True)
            vec.tensor_copy(qhalo[e], ph)

        # ---------------- permute DMAs into conv-image layout ----------------
        # dst (64*hh+ilo, 64*yi+x) <- src (64*ya+ilo, 256*yb + 64*(2ci+hh) + x)
        for ci in (() if ABL == "noconv" else range(2)):
            for hh in range(2):
                for srct, dstt, off in ((sq2, q2q, 64), (sk2, k2q, 0), (sv2, v2q, 0)):
                    for ya in range(2):
                        src = srct[64 * ya:64 * ya + 64, :].rearrange(
                            "i (r h x) -> h i r x", r=8, h=4, x=64)[2 * ci + hh]
                        dst = dstt[ci][64 * hh:64 * hh + 64,
                                       off + 512 * ya:off + 512 * ya + 512
                                       ].rearrange("i (r x) -> i r x", x=64)
                        dma(out=dst, in_=src)
                for e, dlo, dhi in ((0, 0, 64), (1, 1088, 1152)):
                    src = qhalo[e].rearrange("i (h x) -> h i x", h=4)[2 * ci + hh]
                    dma(out=q2q[ci][64 * hh:64 * hh + 64, dlo:dhi], in_=src)

        # release scratch pools; allocate late pools in the freed space
        sctx.close()
        kp = ctx.enter_context(tc.tile_pool(name="late", bufs=1))
        wp = ctx.enter_context(tc.tile_pool(name="work", bufs=2))
        k2l = [kp.tile([128, 1024], FR, tag="k2l0", name="k2l0"),
               kp.tile([128, 1024], FR, tag="k2l1", name="k2l1")]
        k2r = [kp.tile([128, 1024], FR, tag="k2r0", name="k2r0"),
               kp.tile([128, 1024], FR, tag="k2r1", name="k2r1")]

        # k2 shifted-by-x copies with zeroed block edges
        for ci in (() if ABL == "noconv" else range(2)):
            kv = k2q[ci].rearrange("p (y x) -> p y x", x=64)
            vec.tensor_copy(k2l[ci][:, 63::64], zconst.to_broadcast((128, 16)))
            lv = k2l[ci].rearrange("p (y x) -> p y x", x=64)
            vec.tensor_copy(lv[:, :, 0:63], kv[:, :, 1:64])
            vec.tensor_copy(k2r[ci][:, 0::64], zconst.to_broadcast((128, 16)))
            rv = k2r[ci].rearrange("p (y x) -> p y x", x=64)
            vec.tensor_copy(rv[:, :, 1:64], kv[:, :, 0:63])

        # ---------------- conv matmuls + pair-avg + store ----------------
        v2p = [kp.tile([128, 512], FP, tag="v2p0", name="v2p0"),
               kp.tile([128, 512], FP, tag="v2p1", name="v2p1")]
        pavb = [kp.tile([128, 512], FP, tag="pavb0", name="pavb0"),
                kp.tile([128, 512], FP, tag="pavb1", name="pavb1")]
        for oc in (() if ABL == "noconv" else range(2)):
            vv = v2q[oc].rearrange("p (e two) -> p e two", two=2)
            vec.tensor_add(v2p[oc], vv[:, :, 0], vv[:, :, 1])
        for oc in (() if ABL == "noconv" else range(2)):
            for ch in range(2):
                ps = pm.tile([128, 512], FP, tag="m", name=f"cv{oc}_{ch}")
                k = 0
                for dy in range(3):
                    for hf in range(2):
                        nc.tensor.matmul(
                            ps, (w1s[hf][:, 256 * dy + 128 * oc:256 * dy + 128 * oc + 128]),
                            (q2q[hf][:, 512 * ch + 64 * dy:512 * ch + 64 * dy + 512]),
                            start=(k == 0), stop=False, skip_group_check=True)
                        k += 1
                for dx, srcb in ((0, k2r), (1, k2q), (2, k2l)):
                    for hf in range(2):
                        nc.tensor.matmul(
                            ps, (w2s[hf][:, 256 * dx + 128 * oc:256 * dx + 128 * oc + 128]),
                            (srcb[hf][:, 512 * ch:512 * ch + 512]),
                            start=False, stop=(k == 11), skip_group_check=True)
                        k += 1
                cop = wp.tile([128, 512], FP, tag="cop", name=f"cop{oc}{ch}")
                vec.tensor_copy(cop, ps)
                pav = pavb[oc][:, 256 * ch:256 * ch + 256]
                csv = cop.rearrange("p (e two) -> p e two", two=2)
                vec.tensor_add(pav, csv[:, :, 0], csv[:, :, 1])
                vec.tensor_add(pav, pav, v2p[oc][:, 256 * ch:256 * ch + 256])
            dma(out=out_c.rearrange("(o w) e -> o w e", w=4)[
                    128 * oc:128 * oc + 128, :, :],
                in_=pavb[oc].rearrange("p (w e) -> p w e", w=4))

        # ---------------- attention ----------------
        for ic in (() if ABL == "noattn" else range(8)):
            pso = po.tile([65, 512], FP, tag="o", name=f"o{ic}")
            rhs_q = (ut[:, 3 * 512 * ic: 3 * 512 * ic + 1535: 3])
            jb = 0
            for gi, glen in enumerate(GROUPS):
                pool = pa_ if gi % 2 == 0 else pb_
                pss = pool.tile([128, 512 * glen], FP, tag="s", name=f"s{ic}_{gi}")
                for q in range(glen):
                    nc.tensor.matmul(
                        pss[:, 512 * q:512 * q + 512],
                        (ut[:, 3 * 128 * (jb + q) + 1: 3 * 128 * (jb + q) + 383: 3]),
                        rhs_q, start=True, stop=True, skip_group_check=True)
                psb = psbs[gi % 2]
                act.activation(psb[:, 0:512 * glen], pss[:, 0:512 * glen],
                               AF.Exp, scale=0.125)
                for q in range(glen):
                    nc.tensor.matmul(
                        pso, (vsb[:, 65 * (jb + q):65 * (jb + q) + 65]),
                        (psb[:, 512 * q:512 * q + 512]),
                        start=(jb + q == 0), stop=(jb + q == 31),
                        skip_group_check=True)
                jb += glen
            # normalize + pair-avg, via transpose to [i, d]
            osb = osbs[ic % 2]
            resb = resbs[ic % 2]
            vec.tensor_copy(osb, pso)
            for tq in range(4):
                ptr = pm.tile([128, 65], FP, tag="m", name=f"tr{ic}_{tq}")
                nc.tensor.transpose(ptr, osb[:, 128 * tq:128 * tq + 128],
                                    idnf[0:65, 0:65])
                trs = trss[(4 * ic + tq) % 2]
                vec.tensor_copy(trs, ptr)
                rec = recs[(4 * ic + tq) % 2]
                vec.reciprocal(rec, trs[:, 64:65])
                pairs = trs[:, 0:64].rearrange("p (e two) -> p e two", two=2)
                pav2 = pav2s[(4 * ic + tq) % 2]
                vec.tensor_add(pav2, pairs[:, :, 0], pairs[:, :, 1])
                vec.tensor_scalar(resb[:, 32 * tq:32 * tq + 32], pav2, rec,
                                  0.5, OP.mult, OP.mult)
            dma(out=out_a[512 * ic:512 * ic + 512, :].rearrange(
                    "(t p) e -> p t e", t=4),
                in_=resb.rearrange("p (t e) -> p t e", t=4))


# =====================================================================
# Host side
# =====================================================================
_NC_CACHE = None


def _get_nc():
    global _NC_CACHE
    if _NC_CACHE is None:
        _NC_CACHE = build_device_program()
    return _NC_CACHE


def make_in_maps(x, qkv_w, bn_gamma, bn_beta, conv1_w, conv2_w):
    x = np.asarray(x, np.float32)
    WT = np.ascontiguousarray(np.asarray(qkv_w, np.float32).T)   # [256, 1536]
    xT = np.ascontiguousarray(x.transpose(0, 2, 1))              # [2, 256, 4096]
    xts = np.ascontiguousarray(
        np.concatenate([xT[0], xT[1]], axis=1).astype(ml_dtypes.bfloat16))
    w1s = np.ascontiguousarray(
        0.5 * np.asarray(conv1_w, np.float32)[:, :, :, 0].transpose(1, 2, 0)
        .reshape(256, 768))                                      # [i, dy*256+o]
    w2s = np.ascontiguousarray(
        0.5 * np.asarray(conv2_w, np.float32)[:, :, 0, :].transpose(1, 2, 0)
        .reshape(256, 768))
    gbar = np.ascontiguousarray(
        np.stack([np.asarray(bn_gamma, np.float32),
                  np.asarray(bn_beta, np.float32)], axis=1))     # [256, 2]
    idn = np.eye(128, dtype=np.float32)

    ilo = np.arange(64)
    in_maps = []
    for c in range(8):
        b, h = c // 4, c % 4
        head_cols = np.concatenate(
            [512 * g + 64 * h + np.arange(64) for g in range(8)])
        conv_cols = np.concatenate(
            [512 * g + 256 + np.arange(256) for g in range(8)])
        xc = np.ascontiguousarray(
            xT[b][:, np.concatenate([head_cols, conv_cols])])    # [256, 2560]

        # slim conv-proj weights: col (j*8+rho)*128 + 64*mr + ilo
        #   -> WT col (3*rho+j)//8 + 3*(2h+mr) + 24*ilo   (j=2 scaled by 0.5)
        wcg = np.zeros((256, 3072), np.float32)
        for j in range(3):
            sc = 0.5 if j == 2 else 1.0
            for rho in range(8):
                o0 = (3 * rho + j) // 8
                for mr in range(2):
                    cols = o0 + 3 * (2 * h + mr) + 24 * ilo
                    wcg[:, (j * 8 + rho) * 128 + 64 * mr + ilo] = sc * WT[:, cols]
        # halo: lo (rho=7, ya=2h-1): o = 2 + 3*(2h-1) + 24*ilo   (h>=1)
        #       hi (rho=0, ya=2h+2): o = 3*(2h+2) + 24*ilo       (h<=2)
        wch = np.zeros((256, 128), np.float32)
        if h >= 1:
            wch[:, 0:64] = WT[:, 2 + 3 * (2 * h - 1) + 24 * ilo]
        if h <= 2:
            wch[:, 64:128] = WT[:, 3 * (2 * h + 2) + 24 * ilo]

        in_maps.append({
            "xts": xts, "xc": xc, "wq": WT, "wcg": wcg, "wch": wch,
            "w1s": w1s, "w2s": w2s, "gb": gbar, "idn": idn,
        })
    return in_maps


def assemble(results):
    out = np.zeros((B, N, DIM), np.float32)
    for c in range(8):
        b, h = c // 4, c % 4
        out[b, :, 32 * h:32 * h + 32] = results[c]["out_a"]
        oc = results[c]["out_c"].reshape(256, 4, 128)
        out[b].reshape(256, 16, 256)[:, 4 * h:4 * h + 4, 128:256] = oc
    return out


def kernel(**inputs):
    nc = _get_nc()
    in_maps = make_in_maps(**inputs)
    res = bass_utils.run_bass_kernel_spmd(
        nc, in_maps, core_ids=list(range(8)),
        trace=bool(int(os.environ.get("KERNEL_TRACE", "0"))))
    out = assemble(res.results)
    if res.exec_time_ns is not None:
        print(f"HW exec time: {res.exec_time_ns} ns", file=sys.stderr)
        kernel.last_exec_time_ns = res.exec_time_ns
    kernel.last_results = res
    return out


kernel.last_exec_time_ns = None
kernel.last_results = None



# revision 10
# speedup vs baseline: 1.1920x; 1.1920x over previous
"""Trainium2 Bass kernel for nn_Mhsa_47802986004933.

Model (per batch b of 2):
  BN(train-stats)+ReLU -> 1x1 conv qkv (raw .view reinterpret) ->
  4-head attention on heads 0-3  +  conv-mixing (3x1 / 1x3) on heads 4-7 ->
  concat -> kernel-2 avg pool.

Sharding: 8 cores = (batch b in {0,1}) x (h in {0..3}).
  Core c = 4b + h:
    - full 4096x4096 attention for head h of batch b  -> out[b, :, 32h:32h+32]
    - conv y-quarter [16h, 16h+16)                    -> out[b, n%16 in [4h,4h+4), 128:256]
  Communication-free SPMD: BN stats recomputed on every core from the full x.

Key structural identity: with O = W @ xn [1536, 4096] per batch and
U = O.reshape(12288, 512) (u = 8o+g), token n has q = U[3n], k = U[3n+1],
v = U[3n+2].  Attention head h uses U columns [64h, 64h+64); the conv branch
uses columns [256, 512) with image layout q2[i, y, x] =
U[3*(64*(i%64)+y), 256 + 64*(i//64) + x].

All matmuls run as float32r (f32 storage, 1 cycle/row on the PE at N>=256).
"""
import os
import sys
import numpy as np
import ml_dtypes

sys.path.insert(0, "/opt/trn_rl_repo")

import concourse.bass as bass
import concourse.bacc as bacc
import concourse.mybir as mybir
import concourse.tile as tile
from concourse import bass_utils

B, N, DIM, S = 2, 4096, 256, 64
H, DH, INNER = 8, 64, 512
EPS = 1e-5
FP = mybir.dt.float32
FR = mybir.dt.float32r
BF = mybir.dt.bfloat16
AF = mybir.ActivationFunctionType
OP = mybir.AluOpType

# scores exp groups per 512-query chunk: 32 key-blocks split (3,2)*6+(2,)
# glen-3 groups -> exact exp on ACT; glen-2 groups -> fastexp on DVE
GROUPS = [3, 2] * 6 + [2]
LOG2E = 1.4426950408889634
FE_A = 16.0 * LOG2E              # 128*log2e*0.125 applied to raw scores
FE_B = 16248.0 + 12582912.0      # bias-8 + 1.5*2^23 round-to-int magic


def _r(ap):
    return ap.bitcast(FR)


def build_device_program():
    nc = bacc.Bacc(
        "TRN2", target_bir_lowering=False, debug=False, enable_asserts=True,
        num_devices=8,
    )
    xts = nc.dram_tensor("xts", [256, 8192], BF, kind="ExternalInput").ap()
    xc_d = nc.dram_tensor("xc", [256, 2560], FP, kind="ExternalInput").ap()
    wq_d = nc.dram_tensor("wq", [256, 1536], FP, kind="ExternalInput").ap()
    wcg_d = nc.dram_tensor("wcg", [256, 3072], FP, kind="ExternalInput").ap()
    wch_d = nc.dram_tensor("wch", [256, 128], FP, kind="ExternalInput").ap()
    w1s_d = nc.dram_tensor("w1s", [256, 768], FP, kind="ExternalInput").ap()
    w2s_d = nc.dram_tensor("w2s", [256, 768], FP, kind="ExternalInput").ap()
    gb_d = nc.dram_tensor("gb", [256, 2], FP, kind="ExternalInput").ap()
    idn_d = nc.dram_tensor("idn", [128, 128], FP, kind="ExternalInput").ap()
    out_a = nc.dram_tensor("out_a", [4096, 32], FP, kind="ExternalOutput").ap()
    out_c = nc.dram_tensor("out_c", [1024, 128], FP, kind="ExternalOutput").ap()

    with tile.TileContext(nc) as tc:
        _emit(tc, nc, xts, xc_d, wq_d, wcg_d, wch_d, w1s_d, w2s_d, gb_d, idn_d,
              out_a, out_c)
    nc.compile()
    return nc


def _emit(tc, nc, xts, xc_d, wq_d, wcg_d, wch_d, w1s_d, w2s_d, gb_d, idn_d,
          out_a, out_c):
    from contextlib import ExitStack
    ctx = ExitStack()
    with ctx:
        cp = ctx.enter_context(tc.tile_pool(name="const", bufs=1))
        sctx = ExitStack()
        sp = sctx.enter_context(tc.tile_pool(name="scratch", bufs=1))
        xp = sctx.enter_context(tc.tile_pool(name="xload", bufs=2))
        pm = ctx.enter_context(tc.tile_pool(name="ps_m", bufs=2, space="PSUM"))
        pa_ = ctx.enter_context(tc.tile_pool(name="ps_a", bufs=1, space="PSUM"))
        pb_ = ctx.enter_context(tc.tile_pool(name="ps_b", bufs=1, space="PSUM"))
        po = ctx.enter_context(tc.tile_pool(name="ps_o", bufs=1, space="PSUM"))

        # ---------------- persistent SBUF ----------------
        wq = [cp.tile([128, 1536], FR, tag="wq0", name="wq0"),
              cp.tile([128, 1536], FR, tag="wq1", name="wq1")]
        w1s = [cp.tile([128, 768], FR, tag="w1s0", name="w1s0"),
               cp.tile([128, 768], FR, tag="w1s1", name="w1s1")]
        w2s = [cp.tile([128, 768], FR, tag="w2s0", name="w2s0"),
               cp.tile([128, 768], FR, tag="w2s1", name="w2s1")]
        gb = [cp.tile([128, 2], FP, tag="gb0", name="gb0"),
              cp.tile([128, 2], FP, tag="gb1", name="gb1")]
        idn = cp.tile([128, 128], FR, tag="idn", name="idn")
        ut = cp.tile([64, 12288], FR, tag="ut", name="ut")
        vsb = cp.tile([128, 2080], BF, tag="vsb", name="vsb")
        sq2 = sp.tile([128, 2048], FR, tag="sq2", name="sq2")  # slim q (j=0)
        sk2 = sp.tile([128, 2048], FR, tag="sk2", name="sk2")
        sv2 = sp.tile([128, 2048], FR, tag="sv2", name="sv2")
        qhalo = [sp.tile([64, 256], FR, tag="qhalo0", name="qhalo0"),
                 sp.tile([64, 256], FR, tag="qhalo1", name="qhalo1")]
        q2q = [cp.tile([128, 1152], FR, tag="q2q0", name="q2q0"),
               cp.tile([128, 1152], FR, tag="q2q1", name="q2q1")]
        k2q = [cp.tile([128, 1024], FR, tag="k2q0", name="k2q0"),
               cp.tile([128, 1024], FR, tag="k2q1", name="k2q1")]
        v2q = [cp.tile([128, 1024], FR, tag="v2q0", name="v2q0"),
               cp.tile([128, 1024], FR, tag="v2q1", name="v2q1")]

        # transient loads (scratch pool, released before k2l/k2r + work pools)
        wch = [sp.tile([128, 128], FR, tag="wch0", name="wch0"),
               sp.tile([128, 128], FR, tag="wch1", name="wch1")]
        xcld = [cp.tile([128, 2560], FR, tag="xc0", name="xc0"),
                cp.tile([128, 2560], FR, tag="xc1", name="xc1")]
        bnst = [sp.tile([128, 96], FP, tag="bnst0", name="bnst0"),
                sp.tile([128, 96], FP, tag="bnst1", name="bnst1")]

        dma = nc.sync.dma_start
        vec = nc.vector
        act = nc.scalar

        zconst = cp.tile([128, 1], FP, tag="zconst", name="zconst")
        vec.memset(zconst, 0.0)
        nc.const_aps.aps[(FP, 0.0)] = zconst
        epst = cp.tile([128, 1], FP, tag="epst", name="epst")
        vec.memset(epst, EPS)
        onec = cp.tile([128, 1], FP, tag="onec", name="onec")
        vec.memset(onec, 1.0)

        # attention work tiles (declared before scratch release; manual 2x alt)
        psbs = [cp.tile([128, 1536], BF, tag="psbA", name="psbA"),
                cp.tile([128, 1536], BF, tag="psbB", name="psbB")]
        fscs = [cp.tile([128, 1024], FP, tag="fscA", name="fscA"),
                cp.tile([128, 1024], FP, tag="fscB", name="fscB")]
        resbs = [cp.tile([128, 128], FP, tag="resbA", name="resbA"),
                 cp.tile([128, 128], FP, tag="resbB", name="resbB")]
        recs = [cp.tile([128, 1], FP, tag="recA", name="recA"),
                cp.tile([128, 1], FP, tag="recB", name="recB")]
        pav2s = [cp.tile([128, 64], FP, tag="pav2A", name="pav2A"),
                 cp.tile([128, 64], FP, tag="pav2B", name="pav2B")]

        # ---------------- BN stats over full x (critical path: DMA first) ----
        mvs = []
        for hf in range(2):
            for ck in range(2):
                xk = xp.tile([128, 4096], BF, tag="xk", name=f"xk{hf}{ck}")
                dma(out=xk, in_=xts[128 * hf:128 * hf + 128,
                                    4096 * ck:4096 * ck + 4096])
                for sub in range(8):
                    kk = 8 * ck + sub
                    vec.bn_stats(out=bnst[hf][:, 6 * kk:6 * kk + 6],
                                 in_=xk[:, 512 * sub:512 * sub + 512])
            mv = sp.tile([128, 2], FP, tag="mv", bufs=2, name=f"mv{hf}")
            vec.bn_aggr(out=mv, in_=bnst[hf].rearrange("p (k s) -> p k s", s=6))
            mvs.append(mv)

        # ---------------- loads (xc/wq next on the critical path) ----------
        for hf in range(2):
            dma(out=xcld[hf], in_=xc_d.bitcast(FR)[128 * hf:128 * hf + 128, :])
            dma(out=gb[hf], in_=gb_d[128 * hf:128 * hf + 128, :])
        for hf in range(2):
            dma(out=wq[hf], in_=wq_d.bitcast(FR)[128 * hf:128 * hf + 128, :])
        for hf in range(2):
            dma(out=w1s[hf], in_=w1s_d.bitcast(FR)[128 * hf:128 * hf + 128, :])
            dma(out=w2s[hf], in_=w2s_d.bitcast(FR)[128 * hf:128 * hf + 128, :])
            dma(out=wch[hf], in_=wch_d.bitcast(FR)[128 * hf:128 * hf + 128, :])
        dma(out=idn, in_=idn_d.bitcast(FR))

        # affine: a = gamma*rsqrt(var+eps); bb = beta - mean*a
        aff = []
        for hf in range(2):
            sqv = sp.tile([128, 1], FP, tag="sqv", bufs=2, name=f"sqv{hf}")
            act.activation(sqv, mvs[hf][:, 1:2], AF.Sqrt, bias=epst)
            rsv = sp.tile([128, 1], FP, tag="rsv", bufs=2, name=f"rsv{hf}")
            vec.reciprocal(rsv, sqv)
            a_ = sp.tile([128, 1], FP, tag="a_", bufs=2, name=f"a{hf}")
            vec.tensor_tensor(a_, rsv, gb[hf][:, 0:1], OP.mult)
            tmp = sp.tile([128, 1], FP, tag="tmp", bufs=2, name=f"tmp{hf}")
            vec.tensor_tensor(tmp, mvs[hf][:, 0:1], a_, OP.mult)
            bb = sp.tile([128, 1], FP, tag="bb", bufs=2, name=f"bb{hf}")
            vec.tensor_tensor(bb, gb[hf][:, 1:2], tmp, OP.subtract)
            aff.append((a_, bb))

        # xn = relu(a*x + b)   (ACT is idle this early; in-place on the load)
        xn = xcld
        for hf in range(2):
            a_, bb = aff[hf]
            act.activation(xn[hf], xn[hf], AF.Relu, bias=bb, scale=a_)

        # ---------------- head projection -> ut ----------------
        # ut[:, 8o+g] = OUT_g[:, o];  OUT_g = xn_head_g^T @ WT
        for g in range(8):
            for oc in range(3):
                ps = pm.tile([64, 512], FP, tag="m", name=f"pr{g}_{oc}")
                nc.tensor.matmul(ps, (xn[0][:, 64 * g:64 * g + 64]),
                                 (wq[0][:, 512 * oc:512 * oc + 512]),
                                 start=True, stop=False)
                nc.tensor.matmul(ps, (xn[1][:, 64 * g:64 * g + 64]),
                                 (wq[1][:, 512 * oc:512 * oc + 512]),
                                 start=False, stop=True)
                vec.tensor_copy(
                    ut[:, 4096 * oc + g: 4096 * oc + g + 4089: 8], ps)

        # ---------------- V tiles (token-major) + ones column ----------------
        vec.tensor_copy(vsb[:, 64::65], onec.to_broadcast((128, 32)))
        for t in range(32):
            pv = pm.tile([128, 64], FR, tag="m", name=f"vt{t}")
            nc.tensor.transpose(pv, ut[:, 3 * 128 * t + 2: 3 * 128 * t + 384: 3],
                                idn[0:64, 0:64])
            vec.tensor_copy(vsb[:, 65 * t:65 * t + 64], pv)

        ABL = ""  # ablation switch used only during development
        # ---------------- conv-input slim projection ----------------
        # sX[64*mr+ilo, 256*rho+f] = U[3*(8*(8*ilo+2h+mr)+rho)+j, 256+f]
        for j, dst in (() if ABL == "noconv" else ((0, sq2), (1, sk2), (2, sv2))):
            wcgj = [sp.tile([128, 1024], FR, tag="wcgj0", name=f"wcgj0_{j}"),
                    sp.tile([128, 1024], FR, tag="wcgj1", name=f"wcgj1_{j}")]
            for hf in range(2):
                dma(out=wcgj[hf], in_=wcg_d.bitcast(FR)[
                    128 * hf:128 * hf + 128, 1024 * j:1024 * j + 1024])
            for rho in range(8):
                g = (3 * rho + j) % 8
                ps = pm.tile([128, 256], FP, tag="m", name=f"pc{j}_{rho}")
                nc.tensor.matmul(ps, (wcgj[0][:, 128 * rho:128 * rho + 128]),
                                 (xn[0][:, 512 + 256 * g:512 + 256 * g + 256]),
                                 start=True, stop=False)
                nc.tensor.matmul(ps, (wcgj[1][:, 128 * rho:128 * rho + 128]),
                                 (xn[1][:, 512 + 256 * g:512 + 256 * g + 256]),
                                 start=False, stop=True)
                vec.tensor_copy(dst[:, 256 * rho:256 * rho + 256], ps)

        # halo rows (j=0): lo rho=7 g=5 ; hi rho=0 g=0  (separate 64-part tiles)
        for e, wcol, gg in (() if ABL == "noconv" else ((0, 0, 5), (1, 64, 0))):
            ph = pm.tile([64, 256], FP, tag="m", name=f"phalo{e}")
            nc.tensor.matmul(ph, (wch[0][:, wcol:wcol + 64]),
                             (xn[0][:, 512 + 256 * gg:512 + 256 * gg + 256]),
                             start=True, stop=False)
            nc.tensor.matmul(ph, (wch[1][:, wcol:wcol + 64]),
                             (xn[1][:, 512 + 256 * gg:512 + 256 * gg + 256]),
                             start=False, stop=True)
            vec.tensor_copy(qhalo[e], ph)

        # ---------------- permute DMAs into conv-image layout ----------------
        # dst (64*hh+ilo, 64*yi+x) <- src (64*ya+ilo, 256*yb + 64*(2ci+hh) + x)
        for ci in (() if ABL == "noconv" else range(2)):
            for hh in range(2):
                for srct, dstt, off in ((sq2, q2q, 64), (sk2, k2q, 0), (sv2, v2q, 0)):
                    for ya in range(2):
                        src = srct[64 * ya:64 * ya + 64, :].rearrange(
                            "i (r h x) -> h i r x", r=8, h=4, x=64)[2 * ci + hh]
                        dst = dstt[ci][64 * hh:64 * hh + 64,
                                       off + 512 * ya:off + 512 * ya + 512
                                       ].rearrange("i (r x) -> i r x", x=64)
                        dma(out=dst, in_=src)
                for e, dlo, dhi in ((0, 0, 64), (1, 1088, 1152)):
                    src = qhalo[e].rearrange("i (h x) -> h i x", h=4)[2 * ci + hh]
                    dma(out=q2q[ci][64 * hh:64 * hh + 64, dlo:dhi], in_=src)

        # release scratch pools; allocate late pools in the freed space
        sctx.close()
        kp = ctx.enter_context(tc.tile_pool(name="late", bufs=1))
        wp = ctx.enter_context(tc.tile_pool(name="work", bufs=2))
        k2l = [kp.tile([128, 1024], FR, tag="k2l0", name="k2l0"),
               kp.tile([128, 1024], FR, tag="k2l1", name="k2l1")]
        k2r = [kp.tile([128, 1024], FR, tag="k2r0", name="k2r0"),
               kp.tile([128, 1024], FR, tag="k2r1", name="k2r1")]

        # k2 shifted-by-x copies with zeroed block edges
        for ci in (() if ABL == "noconv" else range(2)):
            kv = k2q[ci].rearrange("p (y x) -> p y x", x=64)
            vec.tensor_copy(k2l[ci][:, 63::64], zconst.to_broadcast((128, 16)))
            lv = k2l[ci].rearrange("p (y x) -> p y x", x=64)
            vec.tensor_copy(lv[:, :, 0:63], kv[:, :, 1:64])
            vec.tensor_copy(k2r[ci][:, 0::64], zconst.to_broadcast((128, 16)))
            rv = k2r[ci].rearrange("p (y x) -> p y x", x=64)
            vec.tensor_copy(rv[:, :, 1:64], kv[:, :, 0:63])

        # ---------------- conv matmuls + pair-avg + store ----------------
        v2p = [kp.tile([128, 512], FP, tag="v2p0", name="v2p0"),
               kp.tile([128, 512], FP, tag="v2p1", name="v2p1")]
        pavb = [kp.tile([128, 512], FP, tag="pavb0", name="pavb0"),
                kp.tile([128, 512], FP, tag="pavb1", name="pavb1")]
        for oc in (() if ABL == "noconv" else range(2)):
            vv = v2q[oc].rearrange("p (e two) -> p e two", two=2)
            vec.tensor_add(v2p[oc], vv[:, :, 0], vv[:, :, 1])
        for oc in (() if ABL == "noconv" else range(2)):
            for ch in range(2):
                ps = pm.tile([128, 512], FP, tag="m", name=f"cv{oc}_{ch}")
                k = 0
                for dy in range(3):
                    for hf in range(2):
                        nc.tensor.matmul(
                            ps, (w1s[hf][:, 256 * dy + 128 * oc:256 * dy + 128 * oc + 128]),
                            (q2q[hf][:, 512 * ch + 64 * dy:512 * ch + 64 * dy + 512]),
                            start=(k == 0), stop=False, skip_group_check=True)
                        k += 1
                for dx, srcb in ((0, k2r), (1, k2q), (2, k2l)):
                    for hf in range(2):
                        nc.tensor.matmul(
                            ps, (w2s[hf][:, 256 * dx + 128 * oc:256 * dx + 128 * oc + 128]),
                            (srcb[hf][:, 512 * ch:512 * ch + 512]),
                            start=False, stop=(k == 11), skip_group_check=True)
                        k += 1
                cop = wp.tile([128, 512], FP, tag="cop", name=f"cop{oc}{ch}")
                vec.tensor_copy(cop, ps)
                pav = pavb[oc][:, 256 * ch:256 * ch + 256]
                csv = cop.rearrange("p (e two) -> p e two", two=2)
                vec.tensor_add(pav, csv[:, :, 0], csv[:, :, 1])
                vec.tensor_add(pav, pav, v2p[oc][:, 256 * ch:256 * ch + 256])
            dma(out=out_c.rearrange("(o w) e -> o w e", w=4)[
                    128 * oc:128 * oc + 128, :, :],
                in_=pavb[oc].rearrange("p (w e) -> p w e", w=4))

        # ---------------- attention ----------------
        # scores keys-major -> exp -> AV flipped (P stationary, V moving):
        # pso[:, 128t:128t+65] accumulates [128 queries, 64 dims + denom].
        for ic in (() if ABL == "noattn" else range(8)):
            pso = po.tile([128, 512], FP, tag="o", name=f"o{ic}")
            rhs_q = (ut[:, 3 * 512 * ic: 3 * 512 * ic + 1535: 3])
            jb = 0
            for gi, glen in enumerate(GROUPS):
                pool = pa_ if gi % 2 == 0 else pb_
                pss = pool.tile([128, 512 * glen], FP, tag="s", name=f"s{ic}_{gi}")
                for q in range(glen):
                    nc.tensor.matmul(
                        pss[:, 512 * q:512 * q + 512],
                        (ut[:, 3 * 128 * (jb + q) + 1: 3 * 128 * (jb + q) + 383: 3]),
                        rhs_q, start=True, stop=True, skip_group_check=True)
                on_act = (gi % 2 == 0)
                if on_act:
                    psb = psbs[(gi // 2) % 2]
                    act.activation(psb[:, 0:512 * glen], pss[:, 0:512 * glen],
                                   AF.Exp, scale=0.125)
                    pview = psb
                    pstep = 512
                else:
                    fsc = fscs[(gi // 2) % 2]
                    vec.tensor_scalar(fsc[:, 0:512 * glen], pss[:, 0:512 * glen],
                                      FE_A, FE_B, OP.mult, OP.add)
                    pview = fsc.bitcast(BF)
                    pstep = 1024
                for q in range(glen):
                    j = jb + q
                    for t in range(4):
                        st = 2 if pstep == 1024 else 1
                        nc.tensor.matmul(
                            pso[:, 128 * t:128 * t + 65],
                            (pview[:, pstep * q + st * 128 * t:
                                   pstep * q + st * 128 * t + st * 128:st]),
                            (vsb[:, 65 * j:65 * j + 65]),
                            start=(j == 0 and t == 0), stop=(j == 31),
                            skip_group_check=True)
                jb += glen
            # normalize + pair-avg from [query, dim] psum (PSUM single-read
            # rule: copy dims to SBUF, then pair-add + scale on Pool)
            resb = resbs[ic % 2]
            for tq in range(4):
                rec = recs[(4 * ic + tq) % 2]
                vec.reciprocal(rec, pso[:, 128 * tq + 64:128 * tq + 65])
                osb = pav2s[(4 * ic + tq) % 2]
                vec.tensor_copy(osb, pso[:, 128 * tq:128 * tq + 64])
                pairs = osb.rearrange("p (e two) -> p e two", two=2)
                nc.gpsimd.tensor_add(resb[:, 32 * tq:32 * tq + 32],
                                     pairs[:, :, 0], pairs[:, :, 1])
                nc.gpsimd.tensor_scalar(resb[:, 32 * tq:32 * tq + 32],
                                        resb[:, 32 * tq:32 * tq + 32], rec,
                                        0.5, OP.mult, OP.mult)
            dma(out=out_a[512 * ic:512 * ic + 512, :].rearrange(
                    "(t p) e -> p t e", t=4),
                in_=resb.rearrange("p (t e) -> p t e", t=4))


# =====================================================================
# Host side
# =====================================================================
_NC_CACHE = None


def _get_nc():
    global _NC_CACHE
    if _NC_CACHE is None:
        _NC_CACHE = build_device_program()
    return _NC_CACHE


def make_in_maps(x, qkv_w, bn_gamma, bn_beta, conv1_w, conv2_w):
    x = np.asarray(x, np.float32)
    WT = np.ascontiguousarray(np.asarray(qkv_w, np.float32).T)   # [256, 1536]
    xT = np.ascontiguousarray(x.transpose(0, 2, 1))              # [2, 256, 4096]
    xts = np.ascontiguousarray(
        np.concatenate([xT[0], xT[1]], axis=1).astype(ml_dtypes.bfloat16))
    w1s = np.ascontiguousarray(
        0.5 * np.asarray(conv1_w, np.float32)[:, :, :, 0].transpose(1, 2, 0)
        .reshape(256, 768))                                      # [i, dy*256+o]
    w2s = np.ascontiguousarray(
        0.5 * np.asarray(conv2_w, np.float32)[:, :, 0, :].transpose(1, 2, 0)
        .reshape(256, 768))
    gbar = np.ascontiguousarray(
        np.stack([np.asarray(bn_gamma, np.float32),
                  np.asarray(bn_beta, np.float32)], axis=1))     # [256, 2]
    idn = np.eye(128, dtype=np.float32)

    ilo = np.arange(64)
    in_maps = []
    for c in range(8):
        b, h = c // 4, c % 4
        head_cols = np.concatenate(
            [512 * g + 64 * h + np.arange(64) for g in range(8)])
        conv_cols = np.concatenate(
            [512 * g + 256 + np.arange(256) for g in range(8)])
        xc = np.ascontiguousarray(
            xT[b][:, np.concatenate([head_cols, conv_cols])])    # [256, 2560]

        # slim conv-proj weights: col (j*8+rho)*128 + 64*mr + ilo
        #   -> WT col (3*rho+j)//8 + 3*(2h+mr) + 24*ilo   (j=2 scaled by 0.5)
        wcg = np.zeros((256, 3072), np.float32)
        for j in range(3):
            sc = 0.5 if j == 2 else 1.0
            for rho in range(8):
                o0 = (3 * rho + j) // 8
                for mr in range(2):
                    cols = o0 + 3 * (2 * h + mr) + 24 * ilo
                    wcg[:, (j * 8 + rho) * 128 + 64 * mr + ilo] = sc * WT[:, cols]
        # halo: lo (rho=7, ya=2h-1): o = 2 + 3*(2h-1) + 24*ilo   (h>=1)
        #       hi (rho=0, ya=2h+2): o = 3*(2h+2) + 24*ilo       (h<=2)
        wch = np.zeros((256, 128), np.float32)
        if h >= 1:
            wch[:, 0:64] = WT[:, 2 + 3 * (2 * h - 1) + 24 * ilo]
        if h <= 2:
            wch[:, 64:128] = WT[:, 3 * (2 * h + 2) + 24 * ilo]

        in_maps.append({
            "xts": xts, "xc": xc, "wq": WT, "wcg": wcg, "wch": wch,
            "w1s": w1s, "w2s": w2s, "gb": gbar, "idn": idn,
        })
    return in_maps


def assemble(results):
    out = np.zeros((B, N, DIM), np.float32)
    for c in range(8):
        b, h = c // 4, c % 4
        out[b, :, 32 * h:32 * h + 32] = results[c]["out_a"]
        oc = results[c]["out_c"].reshape(256, 4, 128)
        out[b].reshape(256, 16, 256)[:, 4 * h:4 * h + 4, 128:256] = oc
    return out


def kernel(**inputs):
    nc = _get_nc()
    in_maps = make_in_maps(**inputs)
    res = bass_utils.run_bass_kernel_spmd(
        nc, in_maps, core_ids=list(range(8)),
        trace=bool(int(os.environ.get("KERNEL_TRACE", "0"))))
    out = assemble(res.results)
    if res.exec_time_ns is not None:
        print(f"HW exec time: {res.exec_time_ns} ns", file=sys.stderr)
        kernel.last_exec_time_ns = res.exec_time_ns
    kernel.last_results = res
    return out


kernel.last_exec_time_ns = None
kernel.last_results = None



# revision 21
# speedup vs baseline: 1.2066x; 1.0123x over previous
"""Trainium2 Bass kernel for nn_Mhsa_47802986004933.

Model (per batch b of 2):
  BN(train-stats)+ReLU -> 1x1 conv qkv (raw .view reinterpret) ->
  4-head attention on heads 0-3  +  conv-mixing (3x1 / 1x3) on heads 4-7 ->
  concat -> kernel-2 avg pool.

Sharding: 8 cores = (batch b in {0,1}) x (h in {0..3}).
  Core c = 4b + h:
    - full 4096x4096 attention for head h of batch b  -> out[b, :, 32h:32h+32]
    - conv y-quarter [16h, 16h+16)                    -> out[b, n%16 in [4h,4h+4), 128:256]
  Communication-free SPMD: BN stats recomputed on every core from the full x.

Key structural identity: with O = W @ xn [1536, 4096] per batch and
U = O.reshape(12288, 512) (u = 8o+g), token n has q = U[3n], k = U[3n+1],
v = U[3n+2].  Attention head h uses U columns [64h, 64h+64); the conv branch
uses columns [256, 512) with image layout q2[i, y, x] =
U[3*(64*(i%64)+y), 256 + 64*(i//64) + x].

All matmuls run as float32r (f32 storage, 1 cycle/row on the PE at N>=256).
"""
import os
import sys
import numpy as np
import ml_dtypes

sys.path.insert(0, "/opt/trn_rl_repo")

import concourse.bass as bass
import concourse.bacc as bacc
import concourse.mybir as mybir
import concourse.tile as tile
from concourse import bass_utils

B, N, DIM, S = 2, 4096, 256, 64
H, DH, INNER = 8, 64, 512
EPS = 1e-5
FP = mybir.dt.float32
FR = mybir.dt.float32r
BF = mybir.dt.bfloat16
AF = mybir.ActivationFunctionType
OP = mybir.AluOpType

# scores exp groups per 512-query chunk: 16 uniform glen-2 groups over
# 3 rotating PSUM pools (pipeline depth 3).  10 groups -> exact exp on
# ACT; 6 groups -> fastexp bit-trick on DVE.
GROUPS = [2] * 16
ACT_GROUPS = {0, 2, 3, 5, 6, 8, 9, 11, 12, 14}
LOG2E = 1.4426950408889634
FE_A = 16.0 * LOG2E              # 128*log2e*0.125 applied to raw scores
FE_B = 16248.0 + 12582912.0      # bias-8 + 1.5*2^23 round-to-int magic


def _r(ap):
    return ap.bitcast(FR)


def build_device_program():
    nc = bacc.Bacc(
        "TRN2", target_bir_lowering=False, debug=False, enable_asserts=True,
        num_devices=8,
    )
    xts = nc.dram_tensor("xts", [256, 8192], BF, kind="ExternalInput").ap()
    xc_d = nc.dram_tensor("xc", [256, 2560], FP, kind="ExternalInput").ap()
    wq_d = nc.dram_tensor("wq", [256, 1536], FP, kind="ExternalInput").ap()
    wcg_d = nc.dram_tensor("wcg", [256, 3072], FP, kind="ExternalInput").ap()
    wch_d = nc.dram_tensor("wch", [256, 128], FP, kind="ExternalInput").ap()
    w1s_d = nc.dram_tensor("w1s", [256, 768], FP, kind="ExternalInput").ap()
    w2s_d = nc.dram_tensor("w2s", [256, 768], FP, kind="ExternalInput").ap()
    gb_d = nc.dram_tensor("gb", [256, 2], FP, kind="ExternalInput").ap()
    idn_d = nc.dram_tensor("idn", [128, 128], FP, kind="ExternalInput").ap()
    out_a = nc.dram_tensor("out_a", [4096, 32], FP, kind="ExternalOutput").ap()
    out_c = nc.dram_tensor("out_c", [1024, 128], FP, kind="ExternalOutput").ap()

    with tile.TileContext(nc) as tc:
        _emit(tc, nc, xts, xc_d, wq_d, wcg_d, wch_d, w1s_d, w2s_d, gb_d, idn_d,
              out_a, out_c)
    nc.compile()
    return nc


def _emit(tc, nc, xts, xc_d, wq_d, wcg_d, wch_d, w1s_d, w2s_d, gb_d, idn_d,
          out_a, out_c):
    from contextlib import ExitStack
    ctx = ExitStack()
    with ctx:
        cp = ctx.enter_context(tc.tile_pool(name="const", bufs=1))
        sctx = ExitStack()
        sp = sctx.enter_context(tc.tile_pool(name="scratch", bufs=1))
        xp = sctx.enter_context(tc.tile_pool(name="xload", bufs=2))
        pm = ctx.enter_context(tc.tile_pool(name="ps_m", bufs=1, space="PSUM"))
        pa_ = ctx.enter_context(tc.tile_pool(name="ps_a", bufs=1, space="PSUM"))
        pb_ = ctx.enter_context(tc.tile_pool(name="ps_b", bufs=1, space="PSUM"))
        pc_ = ctx.enter_context(tc.tile_pool(name="ps_c", bufs=1, space="PSUM"))
        po = ctx.enter_context(tc.tile_pool(name="ps_o", bufs=1, space="PSUM"))

        # ---------------- persistent SBUF ----------------
        wq = [cp.tile([128, 1536], FR, tag="wq0", name="wq0"),
              cp.tile([128, 1536], FR, tag="wq1", name="wq1")]
        w1s = [cp.tile([128, 768], FR, tag="w1s0", name="w1s0"),
               cp.tile([128, 768], FR, tag="w1s1", name="w1s1")]
        w2s = [cp.tile([128, 768], FR, tag="w2s0", name="w2s0"),
               cp.tile([128, 768], FR, tag="w2s1", name="w2s1")]
        gb = [cp.tile([128, 2], FP, tag="gb0", name="gb0"),
              cp.tile([128, 2], FP, tag="gb1", name="gb1")]
        idn = cp.tile([128, 128], FR, tag="idn", name="idn")
        ut = cp.tile([64, 12288], FR, tag="ut", name="ut")
        vsb = cp.tile([128, 2080], BF, tag="vsb", name="vsb")
        sq2 = sp.tile([128, 2048], FR, tag="sq2", name="sq2")  # slim q (j=0)
        sk2 = sp.tile([128, 2048], FR, tag="sk2", name="sk2")
        sv2 = sp.tile([128, 2048], FR, tag="sv2", name="sv2")
        qhalo = [sp.tile([64, 256], FR, tag="qhalo0", name="qhalo0"),
                 sp.tile([64, 256], FR, tag="qhalo1", name="qhalo1")]
        q2q = [cp.tile([128, 1152], FR, tag="q2q0", name="q2q0"),
               cp.tile([128, 1152], FR, tag="q2q1", name="q2q1")]
        k2q = [cp.tile([128, 1024], FR, tag="k2q0", name="k2q0"),
               cp.tile([128, 1024], FR, tag="k2q1", name="k2q1")]
        v2q = [cp.tile([128, 1024], FR, tag="v2q0", name="v2q0"),
               cp.tile([128, 1024], FR, tag="v2q1", name="v2q1")]

        # transient loads (scratch pool, released before k2l/k2r + work pools)
        wch = [sp.tile([128, 128], FR, tag="wch0", name="wch0"),
               sp.tile([128, 128], FR, tag="wch1", name="wch1")]
        xcld = [cp.tile([128, 2560], FR, tag="xc0", name="xc0"),
                cp.tile([128, 2560], FR, tag="xc1", name="xc1")]
        bnst = [sp.tile([128, 96], FP, tag="bnst0", name="bnst0"),
                sp.tile([128, 96], FP, tag="bnst1", name="bnst1")]

        dma = nc.sync.dma_start
        vec = nc.vector
        act = nc.scalar

        zconst = cp.tile([128, 1], FP, tag="zconst", name="zconst")
        vec.memset(zconst, 0.0)
        nc.const_aps.aps[(FP, 0.0)] = zconst
        epst = cp.tile([128, 1], FP, tag="epst", name="epst")
        vec.memset(epst, EPS)
        onec = cp.tile([128, 1], FP, tag="onec", name="onec")
        vec.memset(onec, 1.0)

        # attention work tiles (declared before scratch release; manual 2x alt)
        psbs = [cp.tile([128, 1024], BF, tag=f"psb{i}", name=f"psb{i}")
                for i in range(3)]
        fscs = [cp.tile([128, 1024], FP, tag=f"fsc{i}", name=f"fsc{i}")
                for i in range(3)]
        resbs = [cp.tile([128, 128], FP, tag="resbA", name="resbA"),
                 cp.tile([128, 128], FP, tag="resbB", name="resbB")]
        recs = [cp.tile([128, 1], FP, tag="recA", name="recA"),
                cp.tile([128, 1], FP, tag="recB", name="recB")]
        pav2s = [cp.tile([128, 64], FP, tag="pav2A", name="pav2A"),
                 cp.tile([128, 64], FP, tag="pav2B", name="pav2B")]

        # ---------------- BN stats over full x (critical path: DMA first) ----
        mvs = []
        for hf in range(2):
            for ck in range(2):
                xk = xp.tile([128, 4096], BF, tag="xk", name=f"xk{hf}{ck}")
                dma(out=xk, in_=xts[128 * hf:128 * hf + 128,
                                    4096 * ck:4096 * ck + 4096])
                for sub in range(8):
                    kk = 8 * ck + sub
                    vec.bn_stats(out=bnst[hf][:, 6 * kk:6 * kk + 6],
                                 in_=xk[:, 512 * sub:512 * sub + 512])
            mv = sp.tile([128, 2], FP, tag="mv", bufs=2, name=f"mv{hf}")
            vec.bn_aggr(out=mv, in_=bnst[hf].rearrange("p (k s) -> p k s", s=6))
            mvs.append(mv)

        # ---------------- loads (xc/wq next on the critical path) ----------
        for hf in range(2):
            dma(out=xcld[hf], in_=xc_d.bitcast(FR)[128 * hf:128 * hf + 128, :])
            dma(out=gb[hf], in_=gb_d[128 * hf:128 * hf + 128, :])
        for hf in range(2):
            dma(out=wq[hf], in_=wq_d.bitcast(FR)[128 * hf:128 * hf + 128, :])
        for hf in range(2):
            dma(out=w1s[hf], in_=w1s_d.bitcast(FR)[128 * hf:128 * hf + 128, :])
            dma(out=w2s[hf], in_=w2s_d.bitcast(FR)[128 * hf:128 * hf + 128, :])
            dma(out=wch[hf], in_=wch_d.bitcast(FR)[128 * hf:128 * hf + 128, :])
        dma(out=idn, in_=idn_d.bitcast(FR))

        # affine: a = gamma*rsqrt(var+eps); bb = beta - mean*a
        aff = []
        for hf in range(2):
            sqv = sp.tile([128, 1], FP, tag="sqv", bufs=2, name=f"sqv{hf}")
            act.activation(sqv, mvs[hf][:, 1:2], AF.Sqrt, bias=epst)
            rsv = sp.tile([128, 1], FP, tag="rsv", bufs=2, name=f"rsv{hf}")
            vec.reciprocal(rsv, sqv)
            a_ = sp.tile([128, 1], FP, tag="a_", bufs=2, name=f"a{hf}")
            vec.tensor_tensor(a_, rsv, gb[hf][:, 0:1], OP.mult)
            tmp = sp.tile([128, 1], FP, tag="tmp", bufs=2, name=f"tmp{hf}")
            vec.tensor_tensor(tmp, mvs[hf][:, 0:1], a_, OP.mult)
            bb = sp.tile([128, 1], FP, tag="bb", bufs=2, name=f"bb{hf}")
            vec.tensor_tensor(bb, gb[hf][:, 1:2], tmp, OP.subtract)
            aff.append((a_, bb))

        # xn = relu(a*x + b)   (ACT is idle this early; in-place on the load)
        xn = xcld
        for hf in range(2):
            a_, bb = aff[hf]
            act.activation(xn[hf], xn[hf], AF.Relu, bias=bb, scale=a_)

        # ---------------- head projection -> ut ----------------
        # ut[:, 8o+g] = OUT_g[:, o];  OUT_g = xn_head_g^T @ WT
        for g in range(8):
            for oc in range(3):
                ps = pm.tile([64, 512], FP, tag="m", name=f"pr{g}_{oc}")
                nc.tensor.matmul(ps, (xn[0][:, 64 * g:64 * g + 64]),
                                 (wq[0][:, 512 * oc:512 * oc + 512]),
                                 start=True, stop=False)
                nc.tensor.matmul(ps, (xn[1][:, 64 * g:64 * g + 64]),
                                 (wq[1][:, 512 * oc:512 * oc + 512]),
                                 start=False, stop=True)
                dst = ut[:, 4096 * oc + g: 4096 * oc + g + 4089: 8]
                if (g + oc) % 2 == 0:
                    vec.tensor_copy(dst, ps)
                else:
                    act.activation(dst, ps, AF.Copy)

        # ---------------- V tiles (token-major) + ones column ----------------
        vec.tensor_copy(vsb[:, 64::65], onec.to_broadcast((128, 32)))
        for t in range(32):
            pv = pm.tile([128, 64], FR, tag="m", name=f"vt{t}")
            nc.tensor.transpose(pv, ut[:, 3 * 128 * t + 2: 3 * 128 * t + 384: 3],
                                idn[0:64, 0:64])
            if t % 2 == 0:
                vec.tensor_copy(vsb[:, 65 * t:65 * t + 64], pv)
            else:
                act.activation(vsb[:, 65 * t:65 * t + 64], pv, AF.Copy)

        ABL = ""  # ablation switch used only during development
        # ---------------- conv-input slim projection ----------------
        # sX[64*mr+ilo, 256*rho+f] = U[3*(8*(8*ilo+2h+mr)+rho)+j, 256+f]
        for j, dst in (() if ABL == "noconv" else ((0, sq2), (1, sk2), (2, sv2))):
            wcgj = [sp.tile([128, 1024], FR, tag="wcgj0", name=f"wcgj0_{j}"),
                    sp.tile([128, 1024], FR, tag="wcgj1", name=f"wcgj1_{j}")]
            for hf in range(2):
                dma(out=wcgj[hf], in_=wcg_d.bitcast(FR)[
                    128 * hf:128 * hf + 128, 1024 * j:1024 * j + 1024])
            for rho in range(8):
                g = (3 * rho + j) % 8
                ps = pm.tile([128, 256], FP, tag="m", name=f"pc{j}_{rho}")
                nc.tensor.matmul(ps, (wcgj[0][:, 128 * rho:128 * rho + 128]),
                                 (xn[0][:, 512 + 256 * g:512 + 256 * g + 256]),
                                 start=True, stop=False)
                nc.tensor.matmul(ps, (wcgj[1][:, 128 * rho:128 * rho + 128]),
                                 (xn[1][:, 512 + 256 * g:512 + 256 * g + 256]),
                                 start=False, stop=True)
                if rho % 2 == 0:
                    vec.tensor_copy(dst[:, 256 * rho:256 * rho + 256], ps)
                else:
                    act.activation(dst[:, 256 * rho:256 * rho + 256], ps,
                                   AF.Copy)

        # halo rows (j=0): lo rho=7 g=5 ; hi rho=0 g=0  (separate 64-part tiles)
        for e, wcol, gg in (() if ABL == "noconv" else ((0, 0, 5), (1, 64, 0))):
            ph = pm.tile([64, 256], FP, tag="m", name=f"phalo{e}")
            nc.tensor.matmul(ph, (wch[0][:, wcol:wcol + 64]),
                             (xn[0][:, 512 + 256 * gg:512 + 256 * gg + 256]),
                             start=True, stop=False)
            nc.tensor.matmul(ph, (wch[1][:, wcol:wcol + 64]),
                             (xn[1][:, 512 + 256 * gg:512 + 256 * gg + 256]),
                             start=False, stop=True)
            vec.tensor_copy(qhalo[e], ph)

        # ---------------- permute DMAs into conv-image layout ----------------
        # dst (64*hh+ilo, 64*yi+x) <- src (64*ya+ilo, 256*yb + 64*(2ci+hh) + x)
        for ci in (() if ABL == "noconv" else range(2)):
            for hh in range(2):
                for srct, dstt, off in ((sq2, q2q, 64), (sk2, k2q, 0), (sv2, v2q, 0)):
                    for ya in range(2):
                        src = srct[64 * ya:64 * ya + 64, :].rearrange(
                            "i (r h x) -> h i r x", r=8, h=4, x=64)[2 * ci + hh]
                        dst = dstt[ci][64 * hh:64 * hh + 64,
                                       off + 512 * ya:off + 512 * ya + 512
                                       ].rearrange("i (r x) -> i r x", x=64)
                        dma(out=dst, in_=src)
                for e, dlo, dhi in ((0, 0, 64), (1, 1088, 1152)):
                    src = qhalo[e].rearrange("i (h x) -> h i x", h=4)[2 * ci + hh]
                    dma(out=q2q[ci][64 * hh:64 * hh + 64, dlo:dhi], in_=src)

        # release scratch pools; allocate late pools in the freed space
        sctx.close()
        kp = ctx.enter_context(tc.tile_pool(name="late", bufs=1))
        wp = ctx.enter_context(tc.tile_pool(name="work", bufs=2))
        k2l = [kp.tile([128, 1024], FR, tag="k2l0", name="k2l0"),
               kp.tile([128, 1024], FR, tag="k2l1", name="k2l1")]
        k2r = [kp.tile([128, 1024], FR, tag="k2r0", name="k2r0"),
               kp.tile([128, 1024], FR, tag="k2r1", name="k2r1")]

        # k2 shifted-by-x copies with zeroed block edges (SBUF-only -> Pool)
        gps = nc.gpsimd
        for ci in (() if ABL == "noconv" else range(2)):
            kv = k2q[ci].rearrange("p (y x) -> p y x", x=64)
            gps.tensor_copy(k2l[ci][:, 63::64], zconst.to_broadcast((128, 16)))
            lv = k2l[ci].rearrange("p (y x) -> p y x", x=64)
            gps.tensor_copy(lv[:, :, 0:63], kv[:, :, 1:64])
            gps.tensor_copy(k2r[ci][:, 0::64], zconst.to_broadcast((128, 16)))
            rv = k2r[ci].rearrange("p (y x) -> p y x", x=64)
            gps.tensor_copy(rv[:, :, 1:64], kv[:, :, 0:63])

        # ---------------- conv matmuls + pair-avg + store ----------------
        v2p = [kp.tile([128, 512], FP, tag="v2p0", name="v2p0"),
               kp.tile([128, 512], FP, tag="v2p1", name="v2p1")]
        pavb = [kp.tile([128, 512], FP, tag="pavb0", name="pavb0"),
                kp.tile([128, 512], FP, tag="pavb1", name="pavb1")]
        for oc in (() if ABL == "noconv" else range(2)):
            vv = v2q[oc].rearrange("p (e two) -> p e two", two=2)
            gps.tensor_add(v2p[oc], vv[:, :, 0], vv[:, :, 1])
        for oc in (() if ABL == "noconv" else range(2)):
            for ch in range(2):
                ps = pm.tile([128, 512], FP, tag="m", name=f"cv{oc}_{ch}")
                k = 0
                for dy in range(3):
                    for hf in range(2):
                        nc.tensor.matmul(
                            ps, (w1s[hf][:, 256 * dy + 128 * oc:256 * dy + 128 * oc + 128]),
                            (q2q[hf][:, 512 * ch + 64 * dy:512 * ch + 64 * dy + 512]),
                            start=(k == 0), stop=False, skip_group_check=True)
                        k += 1
                for dx, srcb in ((0, k2r), (1, k2q), (2, k2l)):
                    for hf in range(2):
                        nc.tensor.matmul(
                            ps, (w2s[hf][:, 256 * dx + 128 * oc:256 * dx + 128 * oc + 128]),
                            (srcb[hf][:, 512 * ch:512 * ch + 512]),
                            start=False, stop=(k == 11), skip_group_check=True)
                        k += 1
                cop = wp.tile([128, 512], FP, tag="cop", name=f"cop{oc}{ch}")
                if ch % 2 == 0:
                    vec.tensor_copy(cop, ps)
                else:
                    act.activation(cop, ps, AF.Copy)
                pav = pavb[oc][:, 256 * ch:256 * ch + 256]
                csv = cop.rearrange("p (e two) -> p e two", two=2)
                gps.tensor_add(pav, csv[:, :, 0], csv[:, :, 1])
                gps.tensor_add(pav, pav, v2p[oc][:, 256 * ch:256 * ch + 256])
            dma(out=out_c.rearrange("(o w) e -> o w e", w=4)[
                    128 * oc:128 * oc + 128, :, :],
                in_=pavb[oc].rearrange("p (w e) -> p w e", w=4))

        # ---------------- attention ----------------
        # scores keys-major -> exp -> AV flipped (P stationary, V moving):
        # pso[:, 128t:128t+65] accumulates [128 queries, 64 dims + denom].
        def emit_av(pso, gi, glen, jb, pview, pstep):
            st = 2 if pstep == 1024 else 1
            for q in range(glen):
                j = jb + q
                for t in range(4):
                    nc.tensor.matmul(
                        pso[:, 128 * t:128 * t + 65],
                        (pview[:, pstep * q + st * 128 * t:
                               pstep * q + st * 128 * t + st * 128:st]),
                        (vsb[:, 65 * j:65 * j + 65]),
                        start=(j == 0 and t == 0), stop=(j == 31),
                        skip_group_check=True)

        spools = [pa_, pb_, pc_]
        for ic in (() if ABL == "noattn" else range(8)):
            pso = po.tile([128, 512], FP, tag="o", name=f"o{ic}")
            rhs_q = (ut[:, 3 * 512 * ic: 3 * 512 * ic + 1535: 3])
            jb = 0
            pend = []
            for gi, glen in enumerate(GROUPS):
                pool = spools[gi % 3]
                pss = pool.tile([128, 512 * glen], FP, tag="s", name=f"s{ic}_{gi}")
                for q in range(glen):
                    nc.tensor.matmul(
                        pss[:, 512 * q:512 * q + 512],
                        (ut[:, 3 * 128 * (jb + q) + 1: 3 * 128 * (jb + q) + 383: 3]),
                        rhs_q, start=True, stop=True, skip_group_check=True)
                if gi in ACT_GROUPS:
                    psb = psbs[gi % 3]
                    act.activation(psb[:, 0:512 * glen], pss[:, 0:512 * glen],
                                   AF.Exp, scale=0.125)
                    pend.append((gi, glen, jb, psb, 512))
                else:
                    fsc = fscs[gi % 3]
                    vec.tensor_scalar(fsc[:, 0:512 * glen], pss[:, 0:512 * glen],
                                      FE_A, FE_B, OP.mult, OP.add)
                    pend.append((gi, glen, jb, fsc.bitcast(BF), 1024))
                if len(pend) > 2:
                    emit_av(pso, *pend.pop(0))
                jb += glen
            for p in pend:
                emit_av(pso, *p)
            # normalize + pair-avg from [query, dim] psum (PSUM single-read
            # rule: copy dims to SBUF, then pair-add + scale on Pool)
            resb = resbs[ic % 2]
            for tq in range(4):
                rec = recs[(4 * ic + tq) % 2]
                vec.reciprocal(rec, pso[:, 128 * tq + 64:128 * tq + 65])
                osb = pav2s[(4 * ic + tq) % 2]
                act.activation(osb, pso[:, 128 * tq:128 * tq + 64], AF.Copy)
                pairs = osb.rearrange("p (e two) -> p e two", two=2)
                nc.gpsimd.tensor_add(resb[:, 32 * tq:32 * tq + 32],
                                     pairs[:, :, 0], pairs[:, :, 1])
                nc.gpsimd.tensor_scalar(resb[:, 32 * tq:32 * tq + 32],
                                        resb[:, 32 * tq:32 * tq + 32], rec,
                                        0.5, OP.mult, OP.mult)
            dma(out=out_a[512 * ic:512 * ic + 512, :].rearrange(
                    "(t p) e -> p t e", t=4),
                in_=resb.rearrange("p (t e) -> p t e", t=4))


# =====================================================================
# Host side
# =====================================================================
_NC_CACHE = None


def _get_nc():
    global _NC_CACHE
    if _NC_CACHE is None:
        _NC_CACHE = build_device_program()
    return _NC_CACHE


def make_in_maps(x, qkv_w, bn_gamma, bn_beta, conv1_w, conv2_w):
    x = np.asarray(x, np.float32)
    WT = np.ascontiguousarray(np.asarray(qkv_w, np.float32).T)   # [256, 1536]
    xT = np.ascontiguousarray(x.transpose(0, 2, 1))              # [2, 256, 4096]
    xts = np.ascontiguousarray(
        np.concatenate([xT[0], xT[1]], axis=1).astype(ml_dtypes.bfloat16))
    w1s = np.ascontiguousarray(
        0.5 * np.asarray(conv1_w, np.float32)[:, :, :, 0].transpose(1, 2, 0)
        .reshape(256, 768))                                      # [i, dy*256+o]
    w2s = np.ascontiguousarray(
        0.5 * np.asarray(conv2_w, np.float32)[:, :, 0, :].transpose(1, 2, 0)
        .reshape(256, 768))
    gbar = np.ascontiguousarray(
        np.stack([np.asarray(bn_gamma, np.float32),
                  np.asarray(bn_beta, np.float32)], axis=1))     # [256, 2]
    idn = np.eye(128, dtype=np.float32)

    ilo = np.arange(64)
    in_maps = []
    for c in range(8):
        b, h = c // 4, c % 4
        head_cols = np.concatenate(
            [512 * g + 64 * h + np.arange(64) for g in range(8)])
        conv_cols = np.concatenate(
            [512 * g + 256 + np.arange(256) for g in range(8)])
        xc = np.ascontiguousarray(
            xT[b][:, np.concatenate([head_cols, conv_cols])])    # [256, 2560]

        # slim conv-proj weights: col (j*8+rho)*128 + 64*mr + ilo
        #   -> WT col (3*rho+j)//8 + 3*(2h+mr) + 24*ilo   (j=2 scaled by 0.5)
        wcg = np.zeros((256, 3072), np.float32)
        for j in range(3):
            sc = 0.5 if j == 2 else 1.0
            for rho in range(8):
                o0 = (3 * rho + j) // 8
                for mr in range(2):
                    cols = o0 + 3 * (2 * h + mr) + 24 * ilo
                    wcg[:, (j * 8 + rho) * 128 + 64 * mr + ilo] = sc * WT[:, cols]
        # halo: lo (rho=7, ya=2h-1): o = 2 + 3*(2h-1) + 24*ilo   (h>=1)
        #       hi (rho=0, ya=2h+2): o = 3*(2h+2) + 24*ilo       (h<=2)
        wch = np.zeros((256, 128), np.float32)
        if h >= 1:
            wch[:, 0:64] = WT[:, 2 + 3 * (2 * h - 1) + 24 * ilo]
        if h <= 2:
            wch[:, 64:128] = WT[:, 3 * (2 * h + 2) + 24 * ilo]

        in_maps.append({
            "xts": xts, "xc": xc, "wq": WT, "wcg": wcg, "wch": wch,
            "w1s": w1s, "w2s": w2s, "gb": gbar, "idn": idn,
        })
    return in_maps


def assemble(results):
    out = np.zeros((B, N, DIM), np.float32)
    for c in range(8):
        b, h = c // 4, c % 4
        out[b, :, 32 * h:32 * h + 32] = results[c]["out_a"]
        oc = results[c]["out_c"].reshape(256, 4, 128)
        out[b].reshape(256, 16, 256)[:, 4 * h:4 * h + 4, 128:256] = oc
    return out


def kernel(**inputs):
    nc = _get_nc()
    in_maps = make_in_maps(**inputs)
    res = bass_utils.run_bass_kernel_spmd(
        nc, in_maps, core_ids=list(range(8)),
        trace=bool(int(os.environ.get("KERNEL_TRACE", "0"))))
    out = assemble(res.results)
    if res.exec_time_ns is not None:
        print(f"HW exec time: {res.exec_time_ns} ns", file=sys.stderr)
        kernel.last_exec_time_ns = res.exec_time_ns
    kernel.last_results = res
    return out


kernel.last_exec_time_ns = None
kernel.last_results = None

